# revision 15
# baseline (speedup 1.0000x reference)
"""CosmosAttention distributed Bass kernel for 8 TRN2 NeuronCores.

Sharding: 8 cores = 2 batches x 4 head-groups (tensor-parallel over heads,
data-parallel over batch). Each core computes QKV + attention for its 4 heads
of its batch, AllGathers attention outputs within its 4-core batch group, and
computes a 512-wide slice of the output projection.

Shapes (full problem): B=2, S=2048, DM=2048, H=16, HD=128, inner=2048.
Per core: S=2048 tokens, 4 heads, inner slice of 512.
"""

import numpy as np
import ml_dtypes

import concourse.bass as bass
import concourse.mybir as mybir
import concourse.tile as tile
from concourse import bacc
from concourse.bass_utils import run_bass_kernel_spmd

F32 = mybir.dt.float32
BF16 = mybir.dt.bfloat16
BF = ml_dtypes.bfloat16

B, S, DM, H, HD = 2, 2048, 2048, 16, 128
HL = 4           # heads per core
IL = HL * HD     # inner slice per core = 512
ND = DM // 128   # 16 d-tiles
NC = 4           # token chunks of 512
NTB = S // 128   # 16 token blocks
CHUNK = 512
EPS = 1e-6
INV_SQRT_HD = 1.0 / float(np.sqrt(HD))

_CACHED = {}


def _build_nc():
    nc = bacc.Bacc("TRN2", target_bir_lowering=False, debug=False, num_devices=8)

    xT = nc.dram_tensor("xT", [DM, S], BF16, kind="ExternalInput")
    wqT = nc.dram_tensor("wqT", [DM, IL], BF16, kind="ExternalInput")
    wkT = nc.dram_tensor("wkT", [DM, IL], BF16, kind="ExternalInput")
    wvT = nc.dram_tensor("wvT", [DM, IL], BF16, kind="ExternalInput")
    woT = nc.dram_tensor("woT", [DM, IL], BF16, kind="ExternalInput")
    caq = nc.dram_tensor("caq", [HD, S], BF16, kind="ExternalInput")
    cbq = nc.dram_tensor("cbq", [HD, S], BF16, kind="ExternalInput")
    cak = nc.dram_tensor("cak", [HD, S], BF16, kind="ExternalInput")
    cbk = nc.dram_tensor("cbk", [HD, S], BF16, kind="ExternalInput")
    yT = nc.dram_tensor("yT", [IL, S], F32, kind="ExternalOutput")

    xT_v = xT.ap().rearrange("(n p) m -> n p m", p=128)
    wv_v = wvT.ap().rearrange("(n p) m -> n p m", p=128)
    wo_v = woT.ap().rearrange("(n p) m -> n p m", p=128)
    # partition-major views for per-head strided weight loads: [p, d, col]
    wq_pv = wqT.ap().rearrange("(n p) m -> p n m", p=128)
    wk_pv = wkT.ap().rearrange("(n p) m -> p n m", p=128)

    with tile.TileContext(nc) as tc:
        with (
            tc.tile_pool(name="v", bufs=NTB) as v_pool,
            tc.tile_pool(name="qk", bufs=4) as qk_pool,
            tc.tile_pool(name="small", bufs=2) as small_pool,
            tc.tile_pool(name="e", bufs=4) as e_pool,
            tc.tile_pool(name="outT", bufs=2) as out_pool,
            tc.tile_pool(name="const", bufs=1) as const_pool,
            tc.tile_pool(name="w", bufs=ND) as w_pool,
            tc.tile_pool(name="ps", bufs=3, space="PSUM") as ps_pool,
            tc.tile_pool(name="dram", bufs=1, space="DRAM") as dram_pool,
        ):
            # constants
            ones128 = const_pool.tile([128, 128], BF16)
            nc.vector.memset(ones128[:], 1.0)
            eps_tile = const_pool.tile([128, 1], F32)
            nc.vector.memset(eps_tile[:], EPS)

            qk_tiles = {}
            v_sb = [None] * NTB
            out_tiles = []
            ag_outs = []

            # ---------------- Phases 1+2: projections + attention ---------
            with (
                tc.tile_pool(name="xt", bufs=NC * ND) as xt_pool,
                tc.tile_pool(name="wh", bufs=5) as wh_pool,
                tc.tile_pool(name="cab", bufs=4) as cab_pool,
                tc.tile_pool(name="tr", bufs=2) as tr_pool,
            ):
                # resident xT, chunk-granular loads for fast ramp
                xt_cs = []
                for c in range(NC):
                    csl = slice(c * CHUNK, (c + 1) * CHUNK)
                    row = []
                    for d in range(ND):
                        t = xt_pool.tile([128, CHUNK], BF16, tag="xt",
                                         name=f"xt{c}_{d}")
                        nc.sync.dma_start(t[:], xT_v[d][:, csl])
                        row.append(t)
                    xt_cs.append(row)

                ca_q = cab_pool.tile([HD, S], BF16, tag="cab")
                cb_q = cab_pool.tile([HD, S], BF16, tag="cab")
                ca_k = cab_pool.tile([HD, S], BF16, tag="cab")
                cb_k = cab_pool.tile([HD, S], BF16, tag="cab")
                nc.sync.dma_start(ca_q[:], caq.ap())
                nc.sync.dma_start(cb_q[:], cbq.ap())
                nc.sync.dma_start(ca_k[:], cak.ap())
                nc.sync.dma_start(cb_k[:], cbk.ap())

                # ---- V projection (natural layout [tok, hd]) ----
                wv_tiles = []
                for d in range(ND):
                    t = w_pool.tile([128, IL], BF16, tag="w")
                    nc.sync.dma_start(t[:], wv_v[d])
                    wv_tiles.append(t)
                for tb in range(NTB):
                    tsl = slice(tb * 128, (tb + 1) * 128)
                    ps = ps_pool.tile([128, IL], F32, tag="acc", bufs=3)
                    for d in range(ND):
                        nc.tensor.matmul(
                            ps[:],
                            xt_cs[tb // 4][d][:, (tb % 4) * 128:
                                              (tb % 4 + 1) * 128],
                            wv_tiles[d][:],
                            start=(d == 0), stop=(d == ND - 1),
                        )
                    vt = v_pool.tile([128, IL], BF16, tag="v")
                    nc.vector.tensor_copy(vt[:], ps[:])
                    v_sb[tb] = vt

                # ---- prefetch wo through the freed wv slots ----
                wo_tiles = []
                for d in range(ND):
                    t = w_pool.tile([128, IL], BF16, tag="w", name=f"wo{d}")
                    nc.sync.dma_start(t[:], wo_v[d])
                    wo_tiles.append(t)

                # ---- Q/K per head, fused with attention for that head ----
                for h in range(HL):
                    hsl = slice(h * 128, (h + 1) * 128)
                    for name, w_pv, ca_t, cb_t in (
                        ("q", wq_pv, ca_q, cb_q),
                        ("k", wk_pv, ca_k, cb_k),
                    ):
                        # one strided DMA: [128, d, 128] head-slice of wT
                        wh = wh_pool.tile([128, ND, 128], BF16, tag="wh",
                                          name=f"wh_{name}{h}")
                        nc.sync.dma_start(wh[:], w_pv[:, :, hsl])
                        dst = qk_pool.tile([128, S], BF16, tag="qk",
                                           name=f"qk_{name}{h}")
                        qk_tiles[(name, h)] = dst
                        raw = tr_pool.tile([128, S], BF16, tag="raw")
                        for c in range(NC):
                            csl = slice(c * CHUNK, (c + 1) * CHUNK)
                            ps = ps_pool.tile([128, CHUNK], F32, tag="acc",
                                              bufs=3)
                            for d in range(ND):
                                nc.tensor.matmul(
                                    ps[:], wh[:, d, :], xt_cs[c][d][:],
                                    start=(d == 0), stop=(d == ND - 1),
                                )
                            nc.vector.tensor_copy(raw[:, csl], ps[:])
                        # half-swap for rope (cross-partition via DMA)
                        swap = tr_pool.tile([128, S], BF16, tag="swap")
                        nc.sync.dma_start(swap[0:64, :], raw[64:128, :])
                        nc.sync.dma_start(swap[64:128, :], raw[0:64, :])
                        for c in range(NC):
                            csl = slice(c * CHUNK, (c + 1) * CHUNK)
                            # rope: roped = CA*raw + CB*swap
                            r1 = tr_pool.tile([128, CHUNK], BF16, tag="r1")
                            nc.vector.tensor_tensor(
                                r1[:], raw[:, csl], ca_t[:, csl],
                                op=mybir.AluOpType.mult)
                            r2 = tr_pool.tile([128, CHUNK], BF16, tag="r2")
                            nc.vector.tensor_tensor(
                                r2[:], swap[:, csl], cb_t[:, csl],
                                op=mybir.AluOpType.mult)
                            roped = tr_pool.tile([128, CHUNK], BF16, tag="rop")
                            nc.vector.tensor_tensor(
                                roped[:], r1[:], r2[:], op=mybir.AluOpType.add)
                            # rms scale: ss = ones^T raw^2 (bcast over parts)
                            sq = tr_pool.tile([128, CHUNK], BF16, tag="sq")
                            nc.vector.tensor_tensor(
                                sq[:], raw[:, csl], raw[:, csl],
                                op=mybir.AluOpType.mult)
                            ssp = ps_pool.tile([128, CHUNK], F32, tag="acc",
                                               bufs=3)
                            nc.tensor.matmul(ssp[:], ones128[:], sq[:],
                                             start=True, stop=True)
                            sstd = small_pool.tile([128, CHUNK], F32,
                                                   tag="sstd")
                            nc.scalar.activation(
                                sstd[:], ssp[:],
                                mybir.ActivationFunctionType.Sqrt,
                                bias=eps_tile[:], scale=1.0 / HD)
                            rstd = small_pool.tile([128, CHUNK], F32,
                                                   tag="rstd")
                            nc.vector.reciprocal_approx_fast(rstd[:], sstd[:])
                            nc.vector.tensor_tensor(
                                dst[:, csl],
                                roped[:], rstd[:], op=mybir.AluOpType.mult)

                    # ---- attention for head h (interleaves with next h) --
                    qh = qk_tiles[("q", h)]
                    kh = qk_tiles[("k", h)]
                    outT_h = out_pool.tile([128, S], BF16, tag="outT",
                                           name=f"outT{h}")
                    for sc in range(NC):
                        ssl = slice(sc * CHUNK, (sc + 1) * CHUNK)
                        pv = ps_pool.tile([128, CHUNK], F32, tag="pvden",
                                          bufs=2)
                        den = ps_pool.tile([128, CHUNK], F32, tag="pvden",
                                           bufs=2)
                        e_prev = None
                        for tb in range(NTB):
                            sc_ps = ps_pool.tile([128, CHUNK], F32,
                                                 tag="score", bufs=3)
                            nc.tensor.matmul(
                                sc_ps[:], kh[:, tb * 128:(tb + 1) * 128],
                                qh[:, ssl], start=True, stop=True)
                            e = e_pool.tile([128, CHUNK], BF16, tag="e")
                            nc.scalar.activation(
                                e[:], sc_ps[:],
                                mybir.ActivationFunctionType.Exp,
                                bias=0.0, scale=INV_SQRT_HD)
                            nc.tensor.matmul(
                                pv[:], v_sb[tb][:, hsl], e[:],
                                start=(tb == 0), stop=(tb == NTB - 1))
                            if tb % 2 == 0:
                                e_prev = e
                            else:
                                ep = e_pool.tile([128, CHUNK], BF16,
                                                 tag="ep", bufs=3)
                                nc.vector.tensor_tensor(
                                    ep[:], e_prev[:], e[:],
                                    op=mybir.AluOpType.add)
                                nc.tensor.matmul(
                                    den[:], ones128[:], ep[:],
                                    start=(tb == 1), stop=(tb == NTB - 1))
                        rden = small_pool.tile([128, CHUNK], F32, tag="rden")
                        nc.vector.reciprocal_approx_fast(rden[:], den[:])
                        nc.vector.tensor_tensor(
                            outT_h[:, ssl], pv[:], rden[:],
                            op=mybir.AluOpType.mult)
                    out_tiles.append(outT_h)
                    # per-head AllGather (overlaps next head's compute)
                    ag_in = dram_pool.tile([128, S], BF16, tag="agin",
                                           bufs=HL, name=f"agin{h}")
                    nc.gpsimd.dma_start(ag_in[:], outT_h[:])
                    ag_out = dram_pool.tile([IL, S], BF16, tag="agout",
                                            bufs=HL, name=f"agout{h}")
                    nc.gpsimd.collective_compute(
                        "AllGather",
                        mybir.AluOpType.bypass,
                        replica_groups=[[0, 1, 2, 3], [4, 5, 6, 7]],
                        ins=[ag_in.opt()],
                        outs=[ag_out.opt()],
                    )
                    ag_outs.append(ag_out)


            # ---------------- Phase 4: output projection slice ------------
            # inner-dim order is [g][j] = rank j's head g (host permutes w_o)
            with (
                tc.tile_pool(name="ag", bufs=2 * ND + 2) as ag_pool,
            ):
                ag_views = [ag_outs[g].rearrange("(n p) m -> n p m", p=128)
                            for g in range(HL)]
                for c in range(NC):
                    csl = slice(c * CHUNK, (c + 1) * CHUNK)
                    ag_c = []
                    for d in range(ND):
                        g, j = d // 4, d % 4
                        t = ag_pool.tile([128, CHUNK], BF16, tag="ag")
                        nc.sync.dma_start(t[:], ag_views[g][j][:, csl])
                        ag_c.append(t)
                    for mt in range(HL):  # 4 m-tiles of 128
                        msl = slice(mt * 128, (mt + 1) * 128)
                        yp = ps_pool.tile([128, CHUNK], F32, tag="acc",
                                          bufs=3)
                        for d in range(ND):
                            nc.tensor.matmul(
                                yp[:], wo_tiles[d][:, msl], ag_c[d][:],
                                start=(d == 0), stop=(d == ND - 1))
                        y_sb = small_pool.tile([128, CHUNK], F32, tag="ysb")
                        nc.vector.tensor_copy(y_sb[:], yp[:])
                        nc.sync.dma_start(yT.ap()[msl, csl], y_sb[:])

    nc.finalize()
    return nc


# inner-dim permutation for per-head AllGather order:
# block (g, j) of gathered = rank j's local head g = global inner
# [(4*j + g)*128 : (4*j + g + 1)*128]
_WO_PERM = np.concatenate(
    [np.arange(128) + (4 * j + g) * 128 for g in range(4) for j in range(4)])


def _host_prep(x, rope_emb, w_q, w_k, w_v, w_o, q_norm_w, k_norm_w):
    """Build the 8 per-core input maps."""
    f = rope_emb[:, 0].astype(np.float32)  # [S, 64, 2, 2]

    def coeffs(w):
        ca = np.empty((HD, S), np.float32)
        cb = np.empty((HD, S), np.float32)
        ca[0:64] = f[:, :, 0, 0].T * w[0:64, None]
        ca[64:128] = f[:, :, 1, 1].T * w[64:128, None]
        cb[0:64] = f[:, :, 0, 1].T * w[64:128, None]
        cb[64:128] = f[:, :, 1, 0].T * w[0:64, None]
        return ca.astype(BF), cb.astype(BF)

    caq, cbq = coeffs(q_norm_w.astype(np.float32))
    cak, cbk = coeffs(k_norm_w.astype(np.float32))

    in_maps = []
    for c in range(8):
        b, hg = c // 4, c % 4
        sl = slice(IL * hg, IL * (hg + 1))
        in_maps.append({
            "xT": np.ascontiguousarray(x[b].T).astype(BF),
            "wqT": np.ascontiguousarray(w_q[sl].T).astype(BF),
            "wkT": np.ascontiguousarray(w_k[sl].T).astype(BF),
            "wvT": np.ascontiguousarray(w_v[sl].T).astype(BF),
            "woT": np.ascontiguousarray(w_o[sl][:, _WO_PERM].T).astype(BF),
            "caq": caq, "cbq": cbq, "cak": cak, "cbk": cbk,
        })
    return in_maps


def kernel(x, rope_emb, w_q, w_k, w_v, w_o, q_norm_w, k_norm_w, trace=False):
    if "nc" not in _CACHED:
        _CACHED["nc"] = _build_nc()
    nc = _CACHED["nc"]
    in_maps = _host_prep(x, rope_emb, w_q, w_k, w_v, w_o, q_norm_w, k_norm_w)
    res = run_bass_kernel_spmd(nc, in_maps, core_ids=list(range(8)),
                               trace=trace)
    _CACHED["last_result"] = res
    y = np.empty((B, S, DM), np.float32)
    for c in range(8):
        b, hg = c // 4, c % 4
        y[b, :, IL * hg:IL * (hg + 1)] = res.results[c]["yT"].T
    return y


# revision 16
# speedup vs baseline: 1.0353x; 1.0353x over previous
"""CosmosAttention distributed Bass kernel for 8 TRN2 NeuronCores.

Sharding: 8 cores = 2 batches x 4 head-groups (tensor-parallel over heads,
data-parallel over batch). Each core computes QKV + attention for its 4 heads
of its batch, AllGathers attention outputs within its 4-core batch group, and
computes a 512-wide slice of the output projection.

Shapes (full problem): B=2, S=2048, DM=2048, H=16, HD=128, inner=2048.
Per core: S=2048 tokens, 4 heads, inner slice of 512.
"""

import numpy as np
import ml_dtypes

import concourse.bass as bass
import concourse.mybir as mybir
import concourse.tile as tile
from concourse import bacc
from concourse.bass_utils import run_bass_kernel_spmd

F32 = mybir.dt.float32
BF16 = mybir.dt.bfloat16
BF = ml_dtypes.bfloat16

B, S, DM, H, HD = 2, 2048, 2048, 16, 128
HL = 4           # heads per core
IL = HL * HD     # inner slice per core = 512
ND = DM // 128   # 16 d-tiles
NC = 4           # token chunks of 512
NTB = S // 128   # 16 token blocks
CHUNK = 512
EPS = 1e-6
INV_SQRT_HD = 1.0 / float(np.sqrt(HD))

_CACHED = {}


def _build_nc():
    nc = bacc.Bacc("TRN2", target_bir_lowering=False, debug=False, num_devices=8)

    xT = nc.dram_tensor("xT", [DM, S], BF16, kind="ExternalInput")
    wqT = nc.dram_tensor("wqT", [DM, IL], BF16, kind="ExternalInput")
    wkT = nc.dram_tensor("wkT", [DM, IL], BF16, kind="ExternalInput")
    wvT = nc.dram_tensor("wvT", [DM, IL], BF16, kind="ExternalInput")
    woT = nc.dram_tensor("woT", [DM, IL], BF16, kind="ExternalInput")
    caq = nc.dram_tensor("caq", [HD, S], BF16, kind="ExternalInput")
    cbq = nc.dram_tensor("cbq", [HD, S], BF16, kind="ExternalInput")
    cak = nc.dram_tensor("cak", [HD, S], BF16, kind="ExternalInput")
    cbk = nc.dram_tensor("cbk", [HD, S], BF16, kind="ExternalInput")
    yT = nc.dram_tensor("yT", [IL, S], F32, kind="ExternalOutput")

    xT_v = xT.ap().rearrange("(n p) m -> n p m", p=128)
    wv_v = wvT.ap().rearrange("(n p) m -> n p m", p=128)
    wo_v = woT.ap().rearrange("(n p) m -> n p m", p=128)
    # partition-major views for per-head strided weight loads: [p, d, col]
    wq_pv = wqT.ap().rearrange("(n p) m -> p n m", p=128)
    wk_pv = wkT.ap().rearrange("(n p) m -> p n m", p=128)

    with tile.TileContext(nc) as tc:
        with (
            tc.tile_pool(name="v", bufs=NTB) as v_pool,
            tc.tile_pool(name="qk", bufs=4) as qk_pool,
            tc.tile_pool(name="small", bufs=2) as small_pool,
            tc.tile_pool(name="e", bufs=4) as e_pool,
            tc.tile_pool(name="outT", bufs=2) as out_pool,
            tc.tile_pool(name="const", bufs=1) as const_pool,
            tc.tile_pool(name="w", bufs=ND) as w_pool,
            tc.tile_pool(name="ps", bufs=3, space="PSUM") as ps_pool,
            tc.tile_pool(name="dram", bufs=1, space="DRAM") as dram_pool,
        ):
            # constants
            ones128 = const_pool.tile([128, 128], BF16)
            nc.vector.memset(ones128[:], 1.0)
            eps_tile = const_pool.tile([128, 1], F32)
            nc.vector.memset(eps_tile[:], EPS)

            qk_tiles = {}
            v_sb = [None] * NTB
            out_tiles = []
            ag_outs = []

            # ---------------- Phases 1+2: projections + attention ---------
            with (
                tc.tile_pool(name="xt", bufs=ND) as xt_pool,
                tc.tile_pool(name="wh", bufs=5) as wh_pool,
                tc.tile_pool(name="cab", bufs=4) as cab_pool,
                tc.tile_pool(name="tr", bufs=2) as tr_pool,
            ):
                # resident xT
                xt = []
                for d in range(ND):
                    t = xt_pool.tile([128, S], BF16, tag="xt")
                    nc.sync.dma_start(t[:], xT_v[d])
                    xt.append(t)

                ca_q = cab_pool.tile([HD, S], BF16, tag="cab")
                cb_q = cab_pool.tile([HD, S], BF16, tag="cab")
                ca_k = cab_pool.tile([HD, S], BF16, tag="cab")
                cb_k = cab_pool.tile([HD, S], BF16, tag="cab")
                nc.sync.dma_start(ca_q[:], caq.ap())
                nc.sync.dma_start(cb_q[:], cbq.ap())
                nc.sync.dma_start(ca_k[:], cak.ap())
                nc.sync.dma_start(cb_k[:], cbk.ap())

                # ---- V projection (natural layout [tok, hd]) ----
                wv_tiles = []
                for d in range(ND):
                    t = w_pool.tile([128, IL], BF16, tag="w")
                    nc.sync.dma_start(t[:], wv_v[d])
                    wv_tiles.append(t)
                for tb in range(NTB):
                    tsl = slice(tb * 128, (tb + 1) * 128)
                    ps = ps_pool.tile([128, IL], F32, tag="acc", bufs=3)
                    for d in range(ND):
                        nc.tensor.matmul(
                            ps[:], xt[d][:, tsl], wv_tiles[d][:],
                            start=(d == 0), stop=(d == ND - 1),
                        )
                    vt = v_pool.tile([128, IL], BF16, tag="v")
                    nc.vector.tensor_copy(vt[:], ps[:])
                    v_sb[tb] = vt

                # ---- prefetch wo through the freed wv slots ----
                wo_tiles = []
                for d in range(ND):
                    t = w_pool.tile([128, IL], BF16, tag="w", name=f"wo{d}")
                    nc.gpsimd.dma_start(t[:], wo_v[d])
                    wo_tiles.append(t)

                # ---- Q/K per head, fused with attention for that head ----
                for h in range(HL):
                    hsl = slice(h * 128, (h + 1) * 128)
                    for name, w_pv, ca_t, cb_t in (
                        ("q", wq_pv, ca_q, cb_q),
                        ("k", wk_pv, ca_k, cb_k),
                    ):
                        # one strided DMA: [128, d, 128] head-slice of wT
                        wh = wh_pool.tile([128, ND, 128], BF16, tag="wh",
                                          name=f"wh_{name}{h}")
                        nc.sync.dma_start(wh[:], w_pv[:, :, hsl])
                        dst = qk_pool.tile([128, S], BF16, tag="qk",
                                           name=f"qk_{name}{h}")
                        qk_tiles[(name, h)] = dst
                        raw = tr_pool.tile([128, S], BF16, tag="raw")
                        for c in range(NC):
                            csl = slice(c * CHUNK, (c + 1) * CHUNK)
                            ps = ps_pool.tile([128, CHUNK], F32, tag="acc",
                                              bufs=3)
                            for d in range(ND):
                                nc.tensor.matmul(
                                    ps[:], wh[:, d, :], xt[d][:, csl],
                                    start=(d == 0), stop=(d == ND - 1),
                                )
                            nc.vector.tensor_copy(raw[:, csl], ps[:])
                        # half-swap for rope (cross-partition via DMA)
                        swap = tr_pool.tile([128, S], BF16, tag="swap")
                        nc.sync.dma_start(swap[0:64, :], raw[64:128, :])
                        nc.sync.dma_start(swap[64:128, :], raw[0:64, :])
                        for c in range(NC):
                            csl = slice(c * CHUNK, (c + 1) * CHUNK)
                            # rope: roped = CA*raw + CB*swap
                            r1 = tr_pool.tile([128, CHUNK], BF16, tag="r1")
                            nc.vector.tensor_tensor(
                                r1[:], raw[:, csl], ca_t[:, csl],
                                op=mybir.AluOpType.mult)
                            r2 = tr_pool.tile([128, CHUNK], BF16, tag="r2")
                            nc.vector.tensor_tensor(
                                r2[:], swap[:, csl], cb_t[:, csl],
                                op=mybir.AluOpType.mult)
                            roped = tr_pool.tile([128, CHUNK], BF16, tag="rop")
                            nc.vector.tensor_tensor(
                                roped[:], r1[:], r2[:], op=mybir.AluOpType.add)
                            # rms scale: ss = ones^T raw^2 (bcast over parts)
                            sq = tr_pool.tile([128, CHUNK], BF16, tag="sq")
                            nc.vector.tensor_tensor(
                                sq[:], raw[:, csl], raw[:, csl],
                                op=mybir.AluOpType.mult)
                            ssp = ps_pool.tile([128, CHUNK], F32, tag="acc",
                                               bufs=3)
                            nc.tensor.matmul(ssp[:], ones128[:], sq[:],
                                             start=True, stop=True)
                            sstd = small_pool.tile([128, CHUNK], F32,
                                                   tag="sstd")
                            nc.scalar.activation(
                                sstd[:], ssp[:],
                                mybir.ActivationFunctionType.Sqrt,
                                bias=eps_tile[:], scale=1.0 / HD)
                            rstd = small_pool.tile([128, CHUNK], F32,
                                                   tag="rstd")
                            nc.vector.reciprocal_approx_fast(rstd[:], sstd[:])
                            nc.vector.tensor_tensor(
                                dst[:, csl],
                                roped[:], rstd[:], op=mybir.AluOpType.mult)

                    # ---- attention for head h (interleaves with next h) --
                    qh = qk_tiles[("q", h)]
                    kh = qk_tiles[("k", h)]
                    outT_h = out_pool.tile([128, S], BF16, tag="outT",
                                           name=f"outT{h}")
                    for sc in range(NC):
                        ssl = slice(sc * CHUNK, (sc + 1) * CHUNK)
                        pv = ps_pool.tile([128, CHUNK], F32, tag="pvden",
                                          bufs=2)
                        den = ps_pool.tile([128, CHUNK], F32, tag="pvden",
                                           bufs=2)
                        e_prev = None
                        for tb in range(NTB):
                            sc_ps = ps_pool.tile([128, CHUNK], F32,
                                                 tag="score", bufs=3)
                            nc.tensor.matmul(
                                sc_ps[:], kh[:, tb * 128:(tb + 1) * 128],
                                qh[:, ssl], start=True, stop=True)
                            e = e_pool.tile([128, CHUNK], BF16, tag="e")
                            nc.scalar.activation(
                                e[:], sc_ps[:],
                                mybir.ActivationFunctionType.Exp,
                                bias=0.0, scale=INV_SQRT_HD)
                            nc.tensor.matmul(
                                pv[:], v_sb[tb][:, hsl], e[:],
                                start=(tb == 0), stop=(tb == NTB - 1))
                            if tb % 2 == 0:
                                e_prev = e
                            else:
                                ep = e_pool.tile([128, CHUNK], BF16,
                                                 tag="ep", bufs=3)
                                nc.vector.tensor_tensor(
                                    ep[:], e_prev[:], e[:],
                                    op=mybir.AluOpType.add)
                                nc.tensor.matmul(
                                    den[:], ones128[:], ep[:],
                                    start=(tb == 1), stop=(tb == NTB - 1))
                        rden = small_pool.tile([128, CHUNK], F32, tag="rden")
                        nc.vector.reciprocal_approx_fast(rden[:], den[:])
                        nc.vector.tensor_tensor(
                            outT_h[:, ssl], pv[:], rden[:],
                            op=mybir.AluOpType.mult)
                    out_tiles.append(outT_h)
                    # per-head AllGather (overlaps next head's compute)
                    ag_in = dram_pool.tile([128, S], BF16, tag="agin",
                                           bufs=HL, name=f"agin{h}")
                    nc.gpsimd.dma_start(ag_in[:], outT_h[:])
                    ag_out = dram_pool.tile([IL, S], BF16, tag="agout",
                                            bufs=HL, name=f"agout{h}")
                    nc.gpsimd.collective_compute(
                        "AllGather",
                        mybir.AluOpType.bypass,
                        replica_groups=[[0, 1, 2, 3], [4, 5, 6, 7]],
                        ins=[ag_in.opt()],
                        outs=[ag_out.opt()],
                    )
                    ag_outs.append(ag_out)


            # ---------------- Phase 4: output projection slice ------------
            # inner-dim order is [g][j] = rank j's head g (host permutes w_o)
            with (
                tc.tile_pool(name="ag", bufs=2 * ND + 2) as ag_pool,
            ):
                ag_views = [ag_outs[g].rearrange("(n p) m -> n p m", p=128)
                            for g in range(HL)]
                for c in range(NC):
                    csl = slice(c * CHUNK, (c + 1) * CHUNK)
                    ag_c = []
                    for d in range(ND):
                        g, j = d // 4, d % 4
                        t = ag_pool.tile([128, CHUNK], BF16, tag="ag")
                        nc.sync.dma_start(t[:], ag_views[g][j][:, csl])
                        ag_c.append(t)
                    for mt in range(HL):  # 4 m-tiles of 128
                        msl = slice(mt * 128, (mt + 1) * 128)
                        yp = ps_pool.tile([128, CHUNK], F32, tag="acc",
                                          bufs=3)
                        for d in range(ND):
                            nc.tensor.matmul(
                                yp[:], wo_tiles[d][:, msl], ag_c[d][:],
                                start=(d == 0), stop=(d == ND - 1))
                        y_sb = small_pool.tile([128, CHUNK], F32, tag="ysb")
                        nc.vector.tensor_copy(y_sb[:], yp[:])
                        nc.sync.dma_start(yT.ap()[msl, csl], y_sb[:])

    nc.finalize()
    return nc


# inner-dim permutation for per-head AllGather order:
# block (g, j) of gathered = rank j's local head g = global inner
# [(4*j + g)*128 : (4*j + g + 1)*128]
_WO_PERM = np.concatenate(
    [np.arange(128) + (4 * j + g) * 128 for g in range(4) for j in range(4)])


def _host_prep(x, rope_emb, w_q, w_k, w_v, w_o, q_norm_w, k_norm_w):
    """Build the 8 per-core input maps."""
    f = rope_emb[:, 0].astype(np.float32)  # [S, 64, 2, 2]

    def coeffs(w):
        ca = np.empty((HD, S), np.float32)
        cb = np.empty((HD, S), np.float32)
        ca[0:64] = f[:, :, 0, 0].T * w[0:64, None]
        ca[64:128] = f[:, :, 1, 1].T * w[64:128, None]
        cb[0:64] = f[:, :, 0, 1].T * w[64:128, None]
        cb[64:128] = f[:, :, 1, 0].T * w[0:64, None]
        return ca.astype(BF), cb.astype(BF)

    caq, cbq = coeffs(q_norm_w.astype(np.float32))
    cak, cbk = coeffs(k_norm_w.astype(np.float32))

    in_maps = []
    for c in range(8):
        b, hg = c // 4, c % 4
        sl = slice(IL * hg, IL * (hg + 1))
        in_maps.append({
            "xT": np.ascontiguousarray(x[b].T).astype(BF),
            "wqT": np.ascontiguousarray(w_q[sl].T).astype(BF),
            "wkT": np.ascontiguousarray(w_k[sl].T).astype(BF),
            "wvT": np.ascontiguousarray(w_v[sl].T).astype(BF),
            "woT": np.ascontiguousarray(w_o[sl][:, _WO_PERM].T).astype(BF),
            "caq": caq, "cbq": cbq, "cak": cak, "cbk": cbk,
        })
    return in_maps


def kernel(x, rope_emb, w_q, w_k, w_v, w_o, q_norm_w, k_norm_w, trace=False):
    if "nc" not in _CACHED:
        _CACHED["nc"] = _build_nc()
    nc = _CACHED["nc"]
    in_maps = _host_prep(x, rope_emb, w_q, w_k, w_v, w_o, q_norm_w, k_norm_w)
    res = run_bass_kernel_spmd(nc, in_maps, core_ids=list(range(8)),
                               trace=trace)
    _CACHED["last_result"] = res
    y = np.empty((B, S, DM), np.float32)
    for c in range(8):
        b, hg = c // 4, c % 4
        y[b, :, IL * hg:IL * (hg + 1)] = res.results[c]["yT"].T
    return y
